# revision 33
# baseline (speedup 1.0000x reference)
"""Trainium2 Bass kernel for GQA causal self-attention (B=2, T=2048, DIM=2048,
H=16, HKV=4, D=128) with QK-RMSNorm, NTK RoPE, per-head q gain.

Sharding: 8 cores = data-parallel over batch (2) x tensor-parallel over kv
heads (4). Core c handles batch b = c//4, kv-head kv = c%4 (4 q heads).
Per core: qkv projections, attention for its head group, then AllGather of
y^T across the 4-core group and a proj over the full contraction dim for a
512-wide output column shard.

All matmuls run as float32r (TF32-like, ~1e-4 rel err, 4x faster than fp32
on the PE). Softmax is computed without max-subtraction (scores are bounded
by sqrt(D) after QK RMSNorm), with the causal mask applied as a 0/1
multiply on exp() for diagonal blocks and fully-masked blocks skipped.
"""
import numpy as np

B, T, DIM = 2, 2048, 2048
H, HKV, D, G = 16, 4, 128, 4
N_CORES = 8
NTT = T // 128        # 16 t tiles
NDX = DIM // 128      # 16 contraction tiles
QW = G * D            # 512 q dims per core
EPS = float(np.finfo(np.float32).eps)
INV_SQRT_D = 1.0 / float(np.sqrt(D))
ROPE_BASE = 10000.0
TRAIN_SEQ_LEN = 1024

_CACHE = {}


def _rope_tables_T():
    """cosT/sinT [64, T] float32, matching reference._rope_tables exactly."""
    if T > TRAIN_SEQ_LEN:
        sc = T / TRAIN_SEQ_LEN
        base = ROPE_BASE * sc ** (D / (D - 2))
    else:
        base = ROPE_BASE
    inv = 1.0 / base ** (np.arange(0, D, 2, dtype=np.float32) / D)
    f = np.outer(np.arange(T, dtype=np.float32), inv)
    cosT = np.ascontiguousarray(np.cos(f).astype(np.float32).T)
    sinT = np.ascontiguousarray(np.sin(f).astype(np.float32).T)
    return cosT, sinT


def _build(single=False, phases=(1, 2, 3)):
    import concourse.bacc as bacc
    import concourse.mybir as mybir
    import concourse.tile as tile

    F32 = mybir.dt.float32
    F32R = mybir.dt.float32r
    Act = mybir.ActivationFunctionType
    Alu = mybir.AluOpType

    nc = bacc.Bacc("TRN2", target_bir_lowering=False, debug=False,
                   num_devices=1 if single else N_CORES)

    xt = nc.dram_tensor("xt", [DIM, T], F32R, kind="ExternalInput").ap()
    wq = nc.dram_tensor("wq", [128, NDX, QW], F32R, kind="ExternalInput").ap()
    wkv = nc.dram_tensor("wkv", [128, NDX, 256], F32R, kind="ExternalInput").ap()
    wp = nc.dram_tensor("wp", [128, NDX, QW], F32R, kind="ExternalInput").ap()
    cos_d = nc.dram_tensor("cos_d", [128, T], F32, kind="ExternalInput").ap()
    sin_d = nc.dram_tensor("sin_d", [128, T], F32, kind="ExternalInput").ap()
    masks_d = nc.dram_tensor("masks_d", [128, 2048], F32R, kind="ExternalInput").ap()
    ident_d = nc.dram_tensor("ident_d", [128, 128], F32, kind="ExternalInput").ap()
    ones_c_d = nc.dram_tensor("ones_c_d", [128, 1], F32R, kind="ExternalInput").ap()
    ones_r_d = nc.dram_tensor("ones_r_d", [1, 128], F32R, kind="ExternalInput").ap()
    gain_d = nc.dram_tensor("gain_d", [128, G], F32, kind="ExternalInput").ap()
    # per-partition consts for ACT scale/bias (1/D and eps)
    invd_d = nc.dram_tensor("invd_d", [128, 1], F32, kind="ExternalInput").ap()
    eps_d = nc.dram_tensor("eps_d", [128, 1], F32, kind="ExternalInput").ap()

    out_d = nc.dram_tensor("out_sh", [T, QW], F32, kind="ExternalOutput").ap()
    v_d = nc.dram_tensor("v_out", [128, NTT, 128], F32, kind="ExternalOutput").ap()

    groups = [[0, 1, 2, 3], [4, 5, 6, 7]]

    with tile.TileContext(nc) as tc:
        with (
            tc.tile_pool(name="cp", bufs=1) as cp,
            tc.tile_pool(name="dp", bufs=1, space="DRAM") as dp,
        ):
            # long-lived tiles
            qT = cp.tile([128, G, T], F32R)        # roped, normed, scaled q^T
            kT = cp.tile([128, T], F32R)           # roped, normed k^T
            v_r = cp.tile([128, NTT, 128], F32R)   # v in fp32r for PV matmul
            ones_c = cp.tile([128, 1], F32R)
            ones_r = cp.tile([1, 128], F32R)
            gain_sb = cp.tile([128, G], F32)
            invd_sb = cp.tile([128, 1], F32)
            eps_sb = cp.tile([128, 1], F32)
            nc.sync.dma_start(ones_c[:], ones_c_d[:])
            nc.sync.dma_start(ones_r[:], ones_r_d[:])
            nc.sync.dma_start(gain_sb[:], gain_d[:])
            nc.sync.dma_start(invd_sb[:], invd_d[:])
            nc.sync.dma_start(eps_sb[:], eps_d[:])

            y_loc = [dp.tile([128, T], F32R, name=f"yloc{h}") for h in range(G)]
            y_gat = [dp.tile([512, T], F32R, name=f"ygat{h}") for h in range(G)]

            # ---------------- Phase 1: QKV + norm + transpose + RoPE -------
            if 1 in phases:
              with (
                tc.tile_pool(name="p1", bufs=1) as p1,
                tc.tile_pool(name="pp1", bufs=1, space="PSUM") as pp1,
              ):
                wq_sb = p1.tile([128, NDX, QW], F32R)
                wkv_sb = p1.tile([128, NDX, 256], F32R)
                cos_sb = p1.tile([128, T], F32)   # cos duplicated in both halves
                sin_sb = p1.tile([128, T], F32)
                ident = p1.tile([128, 128], F32)
                qTr = p1.tile([128, G, T], F32R)   # transposed q before rope
                kTr = p1.tile([128, T], F32R)      # transposed k before rope
                xt_r = xt.rearrange("(a p) t -> p a t", p=128)

                # prefetch the first two x chunks ahead of the weights
                x_chunks = {}
                for tt in range(2):
                    x_ch = p1.tile([128, NDX, 128], F32R, tag="xch", bufs=2,
                                   name=f"x_ch{tt}")
                    nc.sync.dma_start(x_ch[:],
                                      xt_r[:, :, tt * 128:(tt + 1) * 128])
                    x_chunks[tt] = x_ch
                nc.sync.dma_start(ident[:], ident_d[:])
                for a in range(NDX):   # chunked so first matmuls start early
                    nc.sync.dma_start(wq_sb[:, a, :], wq[:, a, :])
                    nc.sync.dma_start(wkv_sb[:, a, :], wkv[:, a, :])
                nc.sync.dma_start(cos_sb[:], cos_d[:])
                nc.sync.dma_start(sin_sb[:], sin_d[:])


                def rope_chunk(src, dst, c0, c1):
                    # dst[c0:c1] top/bot halves from src[c0:c1] with cos/sin
                    # (cos/sin tables are duplicated across partition halves
                    #  so every 2-input SB op has matching base partitions)
                    ra = p1.tile([64, 512], F32, tag="ra", bufs=2)
                    rb = p1.tile([64, 512], F32, tag="rb", bufs=2)
                    nc.vector.tensor_mul(ra[:], src[0:64, c0:c1],
                                         cos_sb[0:64, c0:c1])
                    nc.vector.tensor_mul(rb[:], src[64:128, c0:c1],
                                         sin_sb[64:128, c0:c1])
                    nc.vector.tensor_add(dst[0:64, c0:c1], ra[:], rb[:])
                    rc = p1.tile([64, 512], F32, tag="ra", bufs=2)
                    rd = p1.tile([64, 512], F32, tag="rb", bufs=2)
                    nc.vector.tensor_mul(rc[:], src[64:128, c0:c1],
                                         cos_sb[64:128, c0:c1])
                    nc.vector.tensor_mul(rd[:], src[0:64, c0:c1],
                                         sin_sb[0:64, c0:c1])
                    nc.vector.tensor_sub(dst[64:128, c0:c1], rc[:], rd[:])

                for tt in range(NTT):
                    if tt in x_chunks:
                        x_ch = x_chunks.pop(tt)
                    else:
                        x_ch = p1.tile([128, NDX, 128], F32R, tag="xch",
                                       bufs=2, name=f"x_ch{tt}")
                        nc.sync.dma_start(
                            x_ch[:], xt_r[:, :, tt * 128:(tt + 1) * 128])
                    psq = pp1.tile([128, QW], F32, tag="psq", bufs=3)
                    pskv = pp1.tile([128, 256], F32, tag="pskv", bufs=3)
                    for a in range(NDX):
                        nc.tensor.matmul(psq[:], x_ch[:, a, :], wq_sb[:, a, :],
                                         start=(a == 0), stop=(a == NDX - 1))
                        nc.tensor.matmul(pskv[:], x_ch[:, a, :], wkv_sb[:, a, :],
                                         start=(a == 0), stop=(a == NDX - 1))

                    # v: fp32 out + fp32r copy for PV (ACT copies from psum)
                    v_t = p1.tile([128, 128], F32, tag="vsb", bufs=2)
                    nc.scalar.activation(v_t[:], pskv[:, 128:256], Act.Copy)
                    nc.sync.dma_start(v_d[:, tt, :], v_t[:])
                    nc.scalar.activation(v_r[:, tt, :], pskv[:, 128:256],
                                         Act.Copy)

                    # q RMSNorm factors: rqg = gain/sqrt(D) / sqrt(mean+eps)
                    sqq = p1.tile([128, QW], F32, tag="sqq", bufs=2)
                    nc.scalar.activation(sqq[:], psq[:], Act.Square)
                    ssq = p1.tile([128, G], F32, tag="ssq", bufs=2)
                    nc.vector.tensor_reduce(
                        ssq[:], sqq[:].rearrange("p (g e) -> p g e", g=G),
                        axis=mybir.AxisListType.X, op=Alu.add)
                    srq = p1.tile([128, G], F32, tag="srq", bufs=2)
                    nc.scalar.activation(srq[:], ssq[:], Act.Sqrt,
                                         scale=invd_sb[:], bias=eps_sb[:])
                    rq = p1.tile([128, G], F32, tag="rq", bufs=2)
                    nc.vector.reciprocal(rq[:], srq[:])
                    rqg = p1.tile([128, G], F32, tag="rqg", bufs=2)
                    nc.vector.tensor_mul(rqg[:], rq[:], gain_sb[:])

                    # k RMSNorm factors
                    sqk = p1.tile([128, 128], F32, tag="sqk", bufs=2)
                    ssk = p1.tile([128, 1], F32, tag="ssk", bufs=2)
                    nc.scalar.activation(sqk[:], pskv[:, 0:128], Act.Square,
                                         accum_out=ssk[:])
                    srk = p1.tile([128, 1], F32, tag="srk", bufs=2)
                    nc.scalar.activation(srk[:], ssk[:], Act.Sqrt,
                                         scale=invd_sb[:], bias=eps_sb[:])
                    rk = p1.tile([128, 1], F32, tag="rk", bufs=2)
                    nc.vector.reciprocal(rk[:], srk[:])

                    # normalize + scale on ACT, transpose on PE
                    qh = p1.tile([128, QW], F32, tag="qh", bufs=2)
                    for g in range(G):
                        nc.scalar.activation(
                            qh[:, g * 128:(g + 1) * 128],
                            psq[:, g * 128:(g + 1) * 128],
                            Act.Copy, scale=rqg[:, g:g + 1])
                    pstq = pp1.tile([128, QW], F32, tag="pstq", bufs=1)
                    for g in range(G):
                        nc.tensor.transpose(pstq[:, g * 128:(g + 1) * 128],
                                            qh[:, g * 128:(g + 1) * 128],
                                            ident[:])
                    nc.vector.tensor_copy(
                        qTr[:, :, tt * 128:(tt + 1) * 128],
                        pstq[:].rearrange("p (g e) -> p g e", g=G))

                    kh = p1.tile([128, 128], F32, tag="kh", bufs=2)
                    nc.scalar.activation(kh[:], pskv[:, 0:128], Act.Copy,
                                         scale=rk[:])
                    pstk = pp1.tile([128, 128], F32, tag="pstk", bufs=1)
                    nc.tensor.transpose(pstk[:], kh[:], ident[:])
                    nc.vector.tensor_copy(kTr[:, tt * 128:(tt + 1) * 128],
                                          pstk[:])

                    # RoPE each finished 512-col chunk (overlaps matmuls)
                    if tt % 4 == 3:
                        c0, c1 = (tt - 3) * 128, (tt + 1) * 128
                        rope_chunk(kTr[:], kT[:], c0, c1)
                        for g in range(G):
                            rope_chunk(qTr[:, g, :], qT[:, g, :], c0, c1)

            # ---------------- Phase 2: attention ---------------------------
            if 2 in phases:
              with (
                tc.tile_pool(name="p2", bufs=1) as p2,
                tc.tile_pool(name="pp2", bufs=1, space="PSUM") as pp2,
              ):
                y_sb = p2.tile([128, G, T], F32R)   # y^T = (P^T V)^T scaled
                mask_sb = p2.tile([128, 2048], F32R)
                nc.sync.dma_start(mask_sb[:], masks_d[:])
                for h in range(G):
                    for ic in range(4):
                        ngroups = ic + 1
                        ps_y = pp2.tile([128, 512], F32, tag="psy", bufs=2)
                        q_sl = qT[:, h, ic * 512:(ic + 1) * 512]
                        den_prev = None
                        for grp in range(ngroups):
                            ex = p2.tile([128, 2048], F32R, tag="ex", bufs=3)
                            for pp in range(2):
                                ps_s = pp2.tile([128, 1024], F32, tag="pss",
                                                bufs=2)
                                for g2 in range(2):
                                    jb = 4 * grp + 2 * pp + g2
                                    nc.tensor.matmul(
                                        ps_s[:, g2 * 512:(g2 + 1) * 512],
                                        kT[:, jb * 128:(jb + 1) * 128], q_sl,
                                        start=True, stop=True)
                                nc.scalar.activation(
                                    ex[:, pp * 1024:(pp + 1) * 1024],
                                    ps_s[:], Act.Exp)
                            exr = ex[:]
                            if grp == ngroups - 1:   # diagonal 4-jb group
                                exm = p2.tile([128, 2048], F32R, tag="exm",
                                              bufs=2)
                                nc.vector.tensor_mul(exm[:], exr, mask_sb[:])
                                exr = exm[:]
                            if den_prev is None:
                                den_prev = exr
                            else:
                                den_t = p2.tile([128, 2048], F32R, tag="den",
                                                bufs=2)
                                nc.vector.tensor_add(den_t[:], den_prev, exr)
                                den_prev = den_t[:]
                            for seg in range(4):
                                jb = 4 * grp + seg
                                nc.tensor.matmul(
                                    ps_y[:], v_r[:, jb, :],
                                    exr[:, seg * 512:(seg + 1) * 512],
                                    start=(grp == 0 and seg == 0),
                                    stop=(grp == ngroups - 1 and seg == 3))
                        # partition-reduce den on PE, recip, broadcast, scale
                        ps_den = pp2.tile([1, 512], F32, tag="psden", bufs=1)
                        for seg in range(4):
                            nc.tensor.matmul(
                                ps_den[:], ones_c[:],
                                den_prev[:, seg * 512:(seg + 1) * 512],
                                start=(seg == 0), stop=(seg == 3))
                        inv_r = p2.tile([1, 512], F32R, tag="invr", bufs=2)
                        with nc.allow_low_precision(reason="fp32r softmax denom"):
                            nc.vector.reciprocal(inv_r[:], ps_den[:])
                        ps_b = pp2.tile([128, 512], F32, tag="psb", bufs=1)
                        nc.tensor.matmul(ps_b[:], ones_r[:], inv_r[:],
                                         start=True, stop=True)
                        invb = p2.tile([128, 512], F32R, tag="invb", bufs=2)
                        nc.scalar.activation(invb[:], ps_b[:], Act.Copy)
                        nc.vector.tensor_mul(
                            y_sb[:, h, ic * 512:(ic + 1) * 512],
                            ps_y[:], invb[:])
                    nc.sync.dma_start(y_loc[h][:], y_sb[:, h, :])
                    if single:
                        # sim-only stand-in: mimic the gather's local traffic
                        for r in range(4):
                            nc.sync.dma_start(
                                y_gat[h][r * 128:(r + 1) * 128, :],
                                y_loc[h][:])
                    else:
                        nc.gpsimd.collective_compute(
                            "AllGather", Alu.bypass, replica_groups=groups,
                            ins=[y_loc[h][:]], outs=[y_gat[h][:]])

            # ---------------- Phase 3: output projection -------------------
            if 3 in phases:
              with (
                tc.tile_pool(name="p3", bufs=1) as p3,
                tc.tile_pool(name="pp3", bufs=1, space="PSUM") as pp3,
              ):
                wp_sb = p3.tile([128, NDX, QW], F32R)
                for a in range(NDX):
                    nc.sync.dma_start(wp_sb[:, a, :], wp[:, a, :])
                for half in range(2):
                    ps_o = [pp3.tile([128, 512], F32, tag=f"o{i}", bufs=1,
                                     name=f"ps_o{half}_{i}")
                            for i in range(8)]
                    # h-major: start accumulating as soon as gather h lands
                    for i, (hs, r) in enumerate(
                            (hs, r) for hs in range(G) for r in range(4)):
                        dyt = 4 * r + hs
                        yblk = p3.tile([128, 1024], F32R, tag="yblk", bufs=3)
                        nc.sync.dma_start(
                            yblk[:],
                            y_gat[hs][r * 128:(r + 1) * 128,
                                      half * 1024:(half + 1) * 1024])
                        for tl in range(8):
                            nc.tensor.matmul(
                                ps_o[tl][:], yblk[:, tl * 128:(tl + 1) * 128],
                                wp_sb[:, dyt, :],
                                start=(i == 0), stop=(i == NDX - 1))
                    for tl in range(8):
                        tt = 8 * half + tl
                        osb = p3.tile([128, 512], F32, tag="osb", bufs=2)
                        nc.vector.tensor_copy(osb[:], ps_o[tl][:])
                        nc.sync.dma_start(out_d[tt * 128:(tt + 1) * 128, :],
                                          osb[:])
    nc.compile()
    return nc


def _get_nc():
    if "nc" not in _CACHE:
        _CACHE["nc"] = _build()
    return _CACHE["nc"]


def _get_runner(nc=None, cache_key="runner"):
    """Build (once) and return (jitted sharded callable, in/out names)."""
    if cache_key in _CACHE:
        return _CACHE[cache_key]

    import jax
    import concourse.mybir as mybir
    from concourse import bass2jax
    from jax.sharding import Mesh, PartitionSpec
    from jax.experimental.shard_map import shard_map

    if nc is None:
        nc = _get_nc()
    bass2jax.install_neuronx_cc_hook()

    partition_name = (nc.partition_id_tensor.name
                      if nc.partition_id_tensor else None)
    in_names, out_names, out_avals, zero_shapes = [], [], [], []
    for alloc in nc.m.functions[0].allocations:
        if not isinstance(alloc, mybir.MemoryLocationSet):
            continue
        name = alloc.memorylocations[0].name
        if alloc.kind == "ExternalInput":
            if name != partition_name:
                in_names.append(name)
        elif alloc.kind == "ExternalOutput":
            out_names.append(name)
            shape = tuple(alloc.tensor_shape)
            dtype = mybir.dt.np(alloc.dtype)
            out_avals.append(jax.core.ShapedArray(shape, dtype))
            zero_shapes.append((shape, dtype))
    n_params = len(in_names)
    n_outs = len(out_avals)
    all_in_names = list(in_names) + list(out_names)
    if partition_name is not None:
        all_in_names.append(partition_name)

    def _body(*args):
        operands = list(args)
        if partition_name is not None:
            operands.append(bass2jax.partition_id_tensor())
        outs = bass2jax._bass_exec_p.bind(
            *operands,
            out_avals=tuple(out_avals),
            in_names=tuple(all_in_names),
            out_names=tuple(out_names),
            lowering_input_output_aliases=(),
            sim_require_finite=True,
            sim_require_nnan=True,
            nc=nc,
        )
        return tuple(outs)

    donate = tuple(range(n_params, n_params + n_outs))
    devices = jax.devices()[:N_CORES]
    mesh = Mesh(np.asarray(devices), ("core",))
    in_specs = (PartitionSpec("core"),) * (n_params + n_outs)
    out_specs = (PartitionSpec("core"),) * n_outs
    sharded = jax.jit(
        shard_map(_body, mesh=mesh, in_specs=in_specs, out_specs=out_specs,
                  check_rep=False),
        donate_argnums=donate, keep_unused=True)
    _CACHE[cache_key] = (sharded, mesh, in_names, out_names, zero_shapes)
    return _CACHE[cache_key]


def _prep_core_maps(x, Wq, Wk, Wv, Wproj, q_gain):
    """Host-side shard prep. Returns list of 8 per-core input dicts."""
    concat = _prep_inputs(x, Wq, Wk, Wv, Wproj, q_gain)
    maps = []
    for c in range(N_CORES):
        m = {}
        for n, arr in concat.items():
            d0 = arr.shape[0] // N_CORES
            m[n] = arr[c * d0:(c + 1) * d0]
        maps.append(m)
    return maps


def _prep_inputs(x, Wq, Wk, Wv, Wproj, q_gain):
    """Host-side shard prep. Returns dict name -> concatenated (8*dim0) array."""
    f = np.float32
    x = np.asarray(x, f)
    Wq = np.asarray(Wq, f)
    Wk = np.asarray(Wk, f)
    Wv = np.asarray(Wv, f)
    Wproj = np.asarray(Wproj, f)
    q_gain = np.asarray(q_gain, f)

    cosT, sinT = _rope_tables_T()
    cosT = np.concatenate([cosT, cosT], axis=0)   # [128, T], both halves
    sinT = np.concatenate([sinT, sinT], axis=0)
    jj = np.arange(128, dtype=np.int32)[:, None]
    ii = np.arange(512, dtype=np.int32)[None, :]
    masks = np.concatenate(
        [(ii >= 128 * r + jj).astype(f) for r in range(4)], axis=1)
    ident = np.eye(128, dtype=f)
    ones_c = np.ones((128, 1), f)
    ones_r = np.ones((1, 128), f)

    xTb = [np.ascontiguousarray(x[b].T) for b in range(B)]

    def wtile(w_rows):  # [rows, DIM] -> [128, NDX, rows] (transposed, tiled)
        wt = np.ascontiguousarray(w_rows.T)          # [DIM, rows]
        return np.ascontiguousarray(
            wt.reshape(NDX, 128, w_rows.shape[0]).transpose(1, 0, 2))

    invd = np.full((128, 1), 1.0 / D, f)
    eps_t = np.full((128, 1), EPS, f)
    per_core = {n: [] for n in ("xt", "wq", "wkv", "wp", "cos_d", "sin_d",
                                "masks_d", "ident_d", "ones_c_d", "ones_r_d",
                                "gain_d", "invd_d", "eps_d")}
    for c in range(N_CORES):
        b, kv = divmod(c, 4)
        wq_s = Wq[kv * QW:(kv + 1) * QW, :]
        wk_s = Wk[kv * D:(kv + 1) * D, :]
        wv_s = Wv[kv * D:(kv + 1) * D, :]
        wkv_s = np.concatenate([wk_s, wv_s], axis=0)    # [256, DIM]
        wp_s = Wproj[kv * QW:(kv + 1) * QW, :]          # output col shard
        gains = np.tile(
            (q_gain[kv * G:(kv + 1) * G] * INV_SQRT_D).astype(f)[None, :],
            (128, 1))
        per_core["xt"].append(xTb[b])
        per_core["wq"].append(wtile(wq_s))
        per_core["wkv"].append(wtile(wkv_s))
        per_core["wp"].append(wtile(wp_s))
        per_core["cos_d"].append(cosT)
        per_core["sin_d"].append(sinT)
        per_core["masks_d"].append(masks)
        per_core["ident_d"].append(ident)
        per_core["ones_c_d"].append(ones_c)
        per_core["ones_r_d"].append(ones_r)
        per_core["gain_d"].append(np.ascontiguousarray(gains))
        per_core["invd_d"].append(invd)
        per_core["eps_d"].append(eps_t)
    return {n: np.concatenate(v, axis=0) for n, v in per_core.items()}


def _assemble(res_out, res_v):
    """res_out: (8, T, QW); res_v: (8, 128, NTT, 128) -> (out, v)."""
    out = np.empty((B, T, DIM), np.float32)
    v = np.empty((B, T, HKV, D), np.float32)
    for c in range(N_CORES):
        b, kv = divmod(c, 4)
        out[b, :, kv * QW:(kv + 1) * QW] = res_out[c]
        v[b, :, kv, :] = res_v[c].transpose(1, 0, 2).reshape(T, D)
    return out, v


def _execute(concat_inputs):
    sharded, mesh, in_names, out_names, zero_shapes = _get_runner()
    args = [concat_inputs[n] for n in in_names]
    zeros = [np.zeros((N_CORES * s[0], *s[1:]), dt) for s, dt in zero_shapes]
    outs = sharded(*args, *zeros)
    result = {}
    for i, name in enumerate(out_names):
        arr = np.asarray(outs[i])
        s = zero_shapes[i][0]
        result[name] = arr.reshape(N_CORES, *s)
    return result


def kernel(x, Wq, Wk, Wv, Wproj, q_gain):
    try:
        concat = _prep_inputs(x, Wq, Wk, Wv, Wproj, q_gain)
        result = _execute(concat)
        return _assemble(result["out_sh"], result["v_out"])
    except Exception:
        # fallback: the stock SPMD runner (handles native NRT and axon)
        from concourse.bass_utils import run_bass_kernel_spmd
        maps = _prep_core_maps(x, Wq, Wk, Wv, Wproj, q_gain)
        res = run_bass_kernel_spmd(_get_nc(), maps,
                                   core_ids=list(range(N_CORES)))
        out_sh = np.stack([res.results[c]["out_sh"] for c in range(N_CORES)])
        v_out = np.stack([res.results[c]["v_out"] for c in range(N_CORES)])
        return _assemble(out_sh, v_out)


# ---------------- benchmarking helpers (used by test.py) --------------------

def _build_noop():
    """Tiny NEFF used to estimate per-call dispatch/RPC overhead."""
    import concourse.bacc as bacc
    import concourse.mybir as mybir
    import concourse.tile as tile
    F32 = mybir.dt.float32
    nc = bacc.Bacc("TRN2", target_bir_lowering=False, debug=False,
                   num_devices=N_CORES)
    a = nc.dram_tensor("na", [128, 128], F32, kind="ExternalInput").ap()
    b = nc.dram_tensor("nb", [128, 128], F32, kind="ExternalOutput").ap()
    with tile.TileContext(nc) as tc:
        with tc.tile_pool(name="sb", bufs=1) as sb:
            t = sb.tile([128, 128], F32)
            nc.sync.dma_start(t[:], a[:])
            nc.sync.dma_start(b[:], t[:])
    nc.compile()
    return nc


def _time_runner(runner_tuple, concat_inputs, iters):
    import jax
    from jax.sharding import NamedSharding, PartitionSpec
    import time as _time
    sharded, mesh, in_names, out_names, zero_shapes = runner_tuple
    shard = NamedSharding(mesh, PartitionSpec("core"))
    dev_args = [jax.device_put(concat_inputs[n], shard) for n in in_names]
    zero_sets = [
        [jax.device_put(np.zeros((N_CORES * s[0], *s[1:]), dt), shard)
         for s, dt in zero_shapes]
        for _ in range(iters + 1)
    ]
    outs = sharded(*dev_args, *zero_sets[0])
    jax.block_until_ready(outs)
    times = []
    for i in range(iters):
        t0 = _time.perf_counter()
        outs = sharded(*dev_args, *zero_sets[i + 1])
        jax.block_until_ready(outs)
        times.append(_time.perf_counter() - t0)
    return times


def bench(concat_inputs, iters=8):
    """Returns (best_kernel_s, best_noop_s, all_kernel_times, all_noop_times)."""
    runner = _get_runner()
    kt = _time_runner(runner, concat_inputs, iters)
    noop_runner = _get_runner(nc=_build_noop(), cache_key="noop_runner")
    nt = _time_runner(noop_runner, {"na": np.zeros(
        (N_CORES * 128, 128), np.float32)}, iters)
    return min(kt), min(nt), kt, nt


# revision 34
# speedup vs baseline: 3.8762x; 3.8762x over previous
"""Trainium2 Bass kernel for GQA causal self-attention (B=2, T=2048, DIM=2048,
H=16, HKV=4, D=128) with QK-RMSNorm, NTK RoPE, per-head q gain.

Sharding: 8 cores = data-parallel over batch (2) x tensor-parallel over kv
heads (4). Core c handles batch b = c//4, kv-head kv = c%4 (4 q heads).
Per core: qkv projections, attention for its head group, then AllGather of
y^T across the 4-core group and a proj over the full contraction dim for a
512-wide output column shard.

All matmuls run as float32r (TF32-like, ~1e-4 rel err, 4x faster than fp32
on the PE). Softmax is computed without max-subtraction (scores are bounded
by sqrt(D) after QK RMSNorm), with the causal mask applied as a 0/1
multiply on exp() for diagonal blocks and fully-masked blocks skipped.
"""
import numpy as np

B, T, DIM = 2, 2048, 2048
H, HKV, D, G = 16, 4, 128, 4
N_CORES = 8
NTT = T // 128        # 16 t tiles
NDX = DIM // 128      # 16 contraction tiles
QW = G * D            # 512 q dims per core
EPS = float(np.finfo(np.float32).eps)
INV_SQRT_D = 1.0 / float(np.sqrt(D))
ROPE_BASE = 10000.0
TRAIN_SEQ_LEN = 1024

_CACHE = {}


def _rope_tables_T():
    """cosT/sinT [64, T] float32, matching reference._rope_tables exactly."""
    if T > TRAIN_SEQ_LEN:
        sc = T / TRAIN_SEQ_LEN
        base = ROPE_BASE * sc ** (D / (D - 2))
    else:
        base = ROPE_BASE
    inv = 1.0 / base ** (np.arange(0, D, 2, dtype=np.float32) / D)
    f = np.outer(np.arange(T, dtype=np.float32), inv)
    cosT = np.ascontiguousarray(np.cos(f).astype(np.float32).T)
    sinT = np.ascontiguousarray(np.sin(f).astype(np.float32).T)
    return cosT, sinT


def _build(single=False, phases=(1, 2, 3)):
    import concourse.bacc as bacc
    import concourse.mybir as mybir
    import concourse.tile as tile

    F32 = mybir.dt.float32
    F32R = mybir.dt.float32r
    Act = mybir.ActivationFunctionType
    Alu = mybir.AluOpType

    nc = bacc.Bacc("TRN2", target_bir_lowering=False, debug=False,
                   num_devices=1 if single else N_CORES)

    xt = nc.dram_tensor("xt", [DIM, T], F32R, kind="ExternalInput").ap()
    wq = nc.dram_tensor("wq", [128, NDX, QW], F32R, kind="ExternalInput").ap()
    wkv = nc.dram_tensor("wkv", [128, NDX, 256], F32R, kind="ExternalInput").ap()
    wp = nc.dram_tensor("wp", [128, NDX, QW], F32R, kind="ExternalInput").ap()
    cos_d = nc.dram_tensor("cos_d", [128, T], F32, kind="ExternalInput").ap()
    sin_d = nc.dram_tensor("sin_d", [128, T], F32, kind="ExternalInput").ap()
    masks_d = nc.dram_tensor("masks_d", [128, 2048], F32R, kind="ExternalInput").ap()
    ident_d = nc.dram_tensor("ident_d", [128, 128], F32, kind="ExternalInput").ap()
    ones_c_d = nc.dram_tensor("ones_c_d", [128, 1], F32R, kind="ExternalInput").ap()
    ones_r_d = nc.dram_tensor("ones_r_d", [1, 128], F32R, kind="ExternalInput").ap()
    gain_d = nc.dram_tensor("gain_d", [128, G], F32, kind="ExternalInput").ap()
    # per-partition consts for ACT scale/bias (1/D and eps)
    invd_d = nc.dram_tensor("invd_d", [128, 1], F32, kind="ExternalInput").ap()
    eps_d = nc.dram_tensor("eps_d", [128, 1], F32, kind="ExternalInput").ap()

    out_d = nc.dram_tensor("out_sh", [T, QW], F32, kind="ExternalOutput").ap()
    v_d = nc.dram_tensor("v_out", [128, NTT, 128], F32, kind="ExternalOutput").ap()

    groups = [[0, 1, 2, 3], [4, 5, 6, 7]]

    with tile.TileContext(nc) as tc:
        with (
            tc.tile_pool(name="cp", bufs=1) as cp,
            tc.tile_pool(name="dp", bufs=1, space="DRAM") as dp,
        ):
            # long-lived tiles
            qT = cp.tile([128, G, T], F32R)        # roped, normed, scaled q^T
            kT = cp.tile([128, T], F32R)           # roped, normed k^T
            v_r = cp.tile([128, NTT, 128], F32R)   # v in fp32r for PV matmul
            ones_c = cp.tile([128, 1], F32R)
            ones_r = cp.tile([1, 128], F32R)
            gain_sb = cp.tile([128, G], F32)
            invd_sb = cp.tile([128, 1], F32)
            eps_sb = cp.tile([128, 1], F32)
            nc.sync.dma_start(ones_c[:], ones_c_d[:])
            nc.sync.dma_start(ones_r[:], ones_r_d[:])
            nc.sync.dma_start(gain_sb[:], gain_d[:])
            nc.sync.dma_start(invd_sb[:], invd_d[:])
            nc.sync.dma_start(eps_sb[:], eps_d[:])

            y_loc = [dp.tile([128, T], F32R, name=f"yloc{h}") for h in range(G)]
            y_gat = [dp.tile([512, T], F32R, name=f"ygat{h}") for h in range(G)]

            # ---------------- Phase 1: QKV + norm + transpose + RoPE -------
            if 1 in phases:
              with (
                tc.tile_pool(name="p1", bufs=1) as p1,
                tc.tile_pool(name="pp1", bufs=1, space="PSUM") as pp1,
              ):
                wq_sb = p1.tile([128, NDX, QW], F32R)
                wkv_sb = p1.tile([128, NDX, 256], F32R)
                cos_sb = p1.tile([128, T], F32)   # cos duplicated in both halves
                sin_sb = p1.tile([128, T], F32)
                ident = p1.tile([128, 128], F32)
                qTr = p1.tile([128, G, T], F32R)   # transposed q before rope
                kTr = p1.tile([128, T], F32R)      # transposed k before rope
                xt_r = xt.rearrange("(a p) t -> p a t", p=128)

                # prefetch the first two x chunks ahead of the weights
                x_chunks = {}
                for tt in range(2):
                    x_ch = p1.tile([128, NDX, 128], F32R, tag="xch", bufs=2,
                                   name=f"x_ch{tt}")
                    nc.sync.dma_start(x_ch[:],
                                      xt_r[:, :, tt * 128:(tt + 1) * 128])
                    x_chunks[tt] = x_ch
                nc.sync.dma_start(ident[:], ident_d[:])
                for a in range(NDX):   # chunked so first matmuls start early
                    nc.sync.dma_start(wq_sb[:, a, :], wq[:, a, :])
                    nc.sync.dma_start(wkv_sb[:, a, :], wkv[:, a, :])
                nc.sync.dma_start(cos_sb[:], cos_d[:])
                nc.sync.dma_start(sin_sb[:], sin_d[:])


                def rope_chunk(src, dst, c0, c1):
                    # dst[c0:c1] top/bot halves from src[c0:c1] with cos/sin
                    # (cos/sin tables are duplicated across partition halves
                    #  so every 2-input SB op has matching base partitions)
                    ra = p1.tile([64, 512], F32, tag="ra", bufs=2)
                    rb = p1.tile([64, 512], F32, tag="rb", bufs=2)
                    nc.vector.tensor_mul(ra[:], src[0:64, c0:c1],
                                         cos_sb[0:64, c0:c1])
                    nc.vector.tensor_mul(rb[:], src[64:128, c0:c1],
                                         sin_sb[64:128, c0:c1])
                    nc.vector.tensor_add(dst[0:64, c0:c1], ra[:], rb[:])
                    rc = p1.tile([64, 512], F32, tag="ra", bufs=2)
                    rd = p1.tile([64, 512], F32, tag="rb", bufs=2)
                    nc.vector.tensor_mul(rc[:], src[64:128, c0:c1],
                                         cos_sb[64:128, c0:c1])
                    nc.vector.tensor_mul(rd[:], src[0:64, c0:c1],
                                         sin_sb[0:64, c0:c1])
                    nc.vector.tensor_sub(dst[64:128, c0:c1], rc[:], rd[:])

                for tt in range(NTT):
                    if tt in x_chunks:
                        x_ch = x_chunks.pop(tt)
                    else:
                        x_ch = p1.tile([128, NDX, 128], F32R, tag="xch",
                                       bufs=2, name=f"x_ch{tt}")
                        nc.sync.dma_start(
                            x_ch[:], xt_r[:, :, tt * 128:(tt + 1) * 128])
                    psq = pp1.tile([128, QW], F32, tag="psq", bufs=3)
                    pskv = pp1.tile([128, 256], F32, tag="pskv", bufs=3)
                    for a in range(NDX):
                        nc.tensor.matmul(psq[:], x_ch[:, a, :], wq_sb[:, a, :],
                                         start=(a == 0), stop=(a == NDX - 1))
                        nc.tensor.matmul(pskv[:], x_ch[:, a, :], wkv_sb[:, a, :],
                                         start=(a == 0), stop=(a == NDX - 1))

                    # v: fp32 out + fp32r copy for PV (ACT copies from psum)
                    v_t = p1.tile([128, 128], F32, tag="vsb", bufs=2)
                    nc.scalar.activation(v_t[:], pskv[:, 128:256], Act.Copy)
                    nc.sync.dma_start(v_d[:, tt, :], v_t[:])
                    nc.scalar.activation(v_r[:, tt, :], pskv[:, 128:256],
                                         Act.Copy)

                    # q RMSNorm factors: rqg = gain/sqrt(D) / sqrt(mean+eps)
                    sqq = p1.tile([128, QW], F32, tag="sqq", bufs=2)
                    nc.scalar.activation(sqq[:], psq[:], Act.Square)
                    ssq = p1.tile([128, G], F32, tag="ssq", bufs=2)
                    nc.vector.tensor_reduce(
                        ssq[:], sqq[:].rearrange("p (g e) -> p g e", g=G),
                        axis=mybir.AxisListType.X, op=Alu.add)
                    srq = p1.tile([128, G], F32, tag="srq", bufs=2)
                    nc.scalar.activation(srq[:], ssq[:], Act.Sqrt,
                                         scale=invd_sb[:], bias=eps_sb[:])
                    rq = p1.tile([128, G], F32, tag="rq", bufs=2)
                    nc.vector.reciprocal(rq[:], srq[:])
                    rqg = p1.tile([128, G], F32, tag="rqg", bufs=2)
                    nc.vector.tensor_mul(rqg[:], rq[:], gain_sb[:])

                    # k RMSNorm factors
                    sqk = p1.tile([128, 128], F32, tag="sqk", bufs=2)
                    ssk = p1.tile([128, 1], F32, tag="ssk", bufs=2)
                    nc.scalar.activation(sqk[:], pskv[:, 0:128], Act.Square,
                                         accum_out=ssk[:])
                    srk = p1.tile([128, 1], F32, tag="srk", bufs=2)
                    nc.scalar.activation(srk[:], ssk[:], Act.Sqrt,
                                         scale=invd_sb[:], bias=eps_sb[:])
                    rk = p1.tile([128, 1], F32, tag="rk", bufs=2)
                    nc.vector.reciprocal(rk[:], srk[:])

                    # normalize + scale on ACT, transpose on PE
                    qh = p1.tile([128, QW], F32, tag="qh", bufs=2)
                    for g in range(G):
                        nc.scalar.activation(
                            qh[:, g * 128:(g + 1) * 128],
                            psq[:, g * 128:(g + 1) * 128],
                            Act.Copy, scale=rqg[:, g:g + 1])
                    pstq = pp1.tile([128, QW], F32, tag="pstq", bufs=1)
                    for g in range(G):
                        nc.tensor.transpose(pstq[:, g * 128:(g + 1) * 128],
                                            qh[:, g * 128:(g + 1) * 128],
                                            ident[:])
                    nc.vector.tensor_copy(
                        qTr[:, :, tt * 128:(tt + 1) * 128],
                        pstq[:].rearrange("p (g e) -> p g e", g=G))

                    kh = p1.tile([128, 128], F32, tag="kh", bufs=2)
                    nc.scalar.activation(kh[:], pskv[:, 0:128], Act.Copy,
                                         scale=rk[:])
                    pstk = pp1.tile([128, 128], F32, tag="pstk", bufs=1)
                    nc.tensor.transpose(pstk[:], kh[:], ident[:])
                    nc.vector.tensor_copy(kTr[:, tt * 128:(tt + 1) * 128],
                                          pstk[:])

                    # RoPE each finished 512-col chunk (overlaps matmuls)
                    if tt % 4 == 3:
                        c0, c1 = (tt - 3) * 128, (tt + 1) * 128
                        rope_chunk(kTr[:], kT[:], c0, c1)
                        for g in range(G):
                            rope_chunk(qTr[:, g, :], qT[:, g, :], c0, c1)

            # ---------------- Phase 2: attention ---------------------------
            if 2 in phases:
              with (
                tc.tile_pool(name="p2", bufs=1) as p2,
                tc.tile_pool(name="pp2", bufs=1, space="PSUM") as pp2,
              ):
                y_sb = p2.tile([128, G, T], F32R)   # y^T = (P^T V)^T scaled
                mask_sb = p2.tile([128, 2048], F32R)
                nc.sync.dma_start(mask_sb[:], masks_d[:])
                # phase-3 weights: prefetched here so the proj can start
                # immediately after the last gather
                wp_sb = p2.tile([128, NDX, QW], F32R)
                for a in range(NDX):
                    nc.sync.dma_start(wp_sb[:, a, :], wp[:, a, :])
                for h in range(G):
                    for ic in range(4):
                        ngroups = ic + 1
                        ps_y = pp2.tile([128, 512], F32, tag="psy", bufs=2)
                        q_sl = qT[:, h, ic * 512:(ic + 1) * 512]
                        den_prev = None
                        for grp in range(ngroups):
                            ex = p2.tile([128, 2048], F32R, tag="ex", bufs=3)
                            for pp in range(2):
                                ps_s = pp2.tile([128, 1024], F32, tag="pss",
                                                bufs=2)
                                for g2 in range(2):
                                    jb = 4 * grp + 2 * pp + g2
                                    nc.tensor.matmul(
                                        ps_s[:, g2 * 512:(g2 + 1) * 512],
                                        kT[:, jb * 128:(jb + 1) * 128], q_sl,
                                        start=True, stop=True)
                                nc.scalar.activation(
                                    ex[:, pp * 1024:(pp + 1) * 1024],
                                    ps_s[:], Act.Exp)
                            exr = ex[:]
                            if grp == ngroups - 1:   # diagonal 4-jb group
                                exm = p2.tile([128, 2048], F32R, tag="exm",
                                              bufs=2)
                                nc.vector.tensor_mul(exm[:], exr, mask_sb[:])
                                exr = exm[:]
                            if den_prev is None:
                                den_prev = exr
                            else:
                                den_t = p2.tile([128, 2048], F32R, tag="den",
                                                bufs=2)
                                nc.vector.tensor_add(den_t[:], den_prev, exr)
                                den_prev = den_t[:]
                            for seg in range(4):
                                jb = 4 * grp + seg
                                nc.tensor.matmul(
                                    ps_y[:], v_r[:, jb, :],
                                    exr[:, seg * 512:(seg + 1) * 512],
                                    start=(grp == 0 and seg == 0),
                                    stop=(grp == ngroups - 1 and seg == 3))
                        # partition-reduce den on PE, recip, broadcast, scale
                        ps_den = pp2.tile([1, 512], F32, tag="psden", bufs=1)
                        for seg in range(4):
                            nc.tensor.matmul(
                                ps_den[:], ones_c[:],
                                den_prev[:, seg * 512:(seg + 1) * 512],
                                start=(seg == 0), stop=(seg == 3))
                        inv_r = p2.tile([1, 512], F32R, tag="invr", bufs=2)
                        with nc.allow_low_precision(reason="fp32r softmax denom"):
                            nc.vector.reciprocal(inv_r[:], ps_den[:])
                        ps_b = pp2.tile([128, 512], F32, tag="psb", bufs=1)
                        nc.tensor.matmul(ps_b[:], ones_r[:], inv_r[:],
                                         start=True, stop=True)
                        invb = p2.tile([128, 512], F32R, tag="invb", bufs=2)
                        nc.scalar.activation(invb[:], ps_b[:], Act.Copy)
                        nc.vector.tensor_mul(
                            y_sb[:, h, ic * 512:(ic + 1) * 512],
                            ps_y[:], invb[:])
                    nc.sync.dma_start(y_loc[h][:], y_sb[:, h, :])
                    if single:
                        # sim-only stand-in: mimic the gather's local traffic
                        for r in range(4):
                            nc.sync.dma_start(
                                y_gat[h][r * 128:(r + 1) * 128, :],
                                y_loc[h][:])
                    else:
                        nc.gpsimd.collective_compute(
                            "AllGather", Alu.bypass, replica_groups=groups,
                            ins=[y_loc[h][:]], outs=[y_gat[h][:]])

            # ---------------- Phase 3: output projection -------------------
            if 3 in phases:
              with (
                tc.tile_pool(name="p3", bufs=1) as p3,
                tc.tile_pool(name="pp3", bufs=1, space="PSUM") as pp3,
              ):
                for half in range(2):
                    ps_o = [pp3.tile([128, 512], F32, tag=f"o{i}", bufs=1,
                                     name=f"ps_o{half}_{i}")
                            for i in range(8)]
                    # h-major: start accumulating as soon as gather h lands
                    for i, (hs, r) in enumerate(
                            (hs, r) for hs in range(G) for r in range(4)):
                        dyt = 4 * r + hs
                        yblk = p3.tile([128, 1024], F32R, tag="yblk", bufs=3)
                        nc.sync.dma_start(
                            yblk[:],
                            y_gat[hs][r * 128:(r + 1) * 128,
                                      half * 1024:(half + 1) * 1024])
                        for tl in range(8):
                            nc.tensor.matmul(
                                ps_o[tl][:], yblk[:, tl * 128:(tl + 1) * 128],
                                wp_sb[:, dyt, :],
                                start=(i == 0), stop=(i == NDX - 1))
                    for tl in range(8):
                        tt = 8 * half + tl
                        osb = p3.tile([128, 512], F32, tag="osb", bufs=2)
                        nc.vector.tensor_copy(osb[:], ps_o[tl][:])
                        nc.sync.dma_start(out_d[tt * 128:(tt + 1) * 128, :],
                                          osb[:])
    nc.compile()
    return nc


def _get_nc():
    if "nc" not in _CACHE:
        _CACHE["nc"] = _build()
    return _CACHE["nc"]


def _get_runner(nc=None, cache_key="runner"):
    """Build (once) and return (jitted sharded callable, in/out names)."""
    if cache_key in _CACHE:
        return _CACHE[cache_key]

    import jax
    import concourse.mybir as mybir
    from concourse import bass2jax
    from jax.sharding import Mesh, PartitionSpec
    from jax.experimental.shard_map import shard_map

    if nc is None:
        nc = _get_nc()
    bass2jax.install_neuronx_cc_hook()

    partition_name = (nc.partition_id_tensor.name
                      if nc.partition_id_tensor else None)
    in_names, out_names, out_avals, zero_shapes = [], [], [], []
    for alloc in nc.m.functions[0].allocations:
        if not isinstance(alloc, mybir.MemoryLocationSet):
            continue
        name = alloc.memorylocations[0].name
        if alloc.kind == "ExternalInput":
            if name != partition_name:
                in_names.append(name)
        elif alloc.kind == "ExternalOutput":
            out_names.append(name)
            shape = tuple(alloc.tensor_shape)
            dtype = mybir.dt.np(alloc.dtype)
            out_avals.append(jax.core.ShapedArray(shape, dtype))
            zero_shapes.append((shape, dtype))
    n_params = len(in_names)
    n_outs = len(out_avals)
    all_in_names = list(in_names) + list(out_names)
    if partition_name is not None:
        all_in_names.append(partition_name)

    def _body(*args):
        operands = list(args)
        if partition_name is not None:
            operands.append(bass2jax.partition_id_tensor())
        outs = bass2jax._bass_exec_p.bind(
            *operands,
            out_avals=tuple(out_avals),
            in_names=tuple(all_in_names),
            out_names=tuple(out_names),
            lowering_input_output_aliases=(),
            sim_require_finite=True,
            sim_require_nnan=True,
            nc=nc,
        )
        return tuple(outs)

    donate = tuple(range(n_params, n_params + n_outs))
    devices = jax.devices()[:N_CORES]
    mesh = Mesh(np.asarray(devices), ("core",))
    in_specs = (PartitionSpec("core"),) * (n_params + n_outs)
    out_specs = (PartitionSpec("core"),) * n_outs
    sharded = jax.jit(
        shard_map(_body, mesh=mesh, in_specs=in_specs, out_specs=out_specs,
                  check_rep=False),
        donate_argnums=donate, keep_unused=True)
    _CACHE[cache_key] = (sharded, mesh, in_names, out_names, zero_shapes)
    return _CACHE[cache_key]


def _prep_core_maps(x, Wq, Wk, Wv, Wproj, q_gain):
    """Host-side shard prep. Returns list of 8 per-core input dicts."""
    concat = _prep_inputs(x, Wq, Wk, Wv, Wproj, q_gain)
    maps = []
    for c in range(N_CORES):
        m = {}
        for n, arr in concat.items():
            d0 = arr.shape[0] // N_CORES
            m[n] = arr[c * d0:(c + 1) * d0]
        maps.append(m)
    return maps


def _prep_inputs(x, Wq, Wk, Wv, Wproj, q_gain):
    """Host-side shard prep. Returns dict name -> concatenated (8*dim0) array."""
    f = np.float32
    x = np.asarray(x, f)
    Wq = np.asarray(Wq, f)
    Wk = np.asarray(Wk, f)
    Wv = np.asarray(Wv, f)
    Wproj = np.asarray(Wproj, f)
    q_gain = np.asarray(q_gain, f)

    cosT, sinT = _rope_tables_T()
    cosT = np.concatenate([cosT, cosT], axis=0)   # [128, T], both halves
    sinT = np.concatenate([sinT, sinT], axis=0)
    jj = np.arange(128, dtype=np.int32)[:, None]
    ii = np.arange(512, dtype=np.int32)[None, :]
    masks = np.concatenate(
        [(ii >= 128 * r + jj).astype(f) for r in range(4)], axis=1)
    ident = np.eye(128, dtype=f)
    ones_c = np.ones((128, 1), f)
    ones_r = np.ones((1, 128), f)

    xTb = [np.ascontiguousarray(x[b].T) for b in range(B)]

    def wtile(w_rows):  # [rows, DIM] -> [128, NDX, rows] (transposed, tiled)
        wt = np.ascontiguousarray(w_rows.T)          # [DIM, rows]
        return np.ascontiguousarray(
            wt.reshape(NDX, 128, w_rows.shape[0]).transpose(1, 0, 2))

    invd = np.full((128, 1), 1.0 / D, f)
    eps_t = np.full((128, 1), EPS, f)
    per_core = {n: [] for n in ("xt", "wq", "wkv", "wp", "cos_d", "sin_d",
                                "masks_d", "ident_d", "ones_c_d", "ones_r_d",
                                "gain_d", "invd_d", "eps_d")}
    for c in range(N_CORES):
        b, kv = divmod(c, 4)
        wq_s = Wq[kv * QW:(kv + 1) * QW, :]
        wk_s = Wk[kv * D:(kv + 1) * D, :]
        wv_s = Wv[kv * D:(kv + 1) * D, :]
        wkv_s = np.concatenate([wk_s, wv_s], axis=0)    # [256, DIM]
        wp_s = Wproj[kv * QW:(kv + 1) * QW, :]          # output col shard
        gains = np.tile(
            (q_gain[kv * G:(kv + 1) * G] * INV_SQRT_D).astype(f)[None, :],
            (128, 1))
        per_core["xt"].append(xTb[b])
        per_core["wq"].append(wtile(wq_s))
        per_core["wkv"].append(wtile(wkv_s))
        per_core["wp"].append(wtile(wp_s))
        per_core["cos_d"].append(cosT)
        per_core["sin_d"].append(sinT)
        per_core["masks_d"].append(masks)
        per_core["ident_d"].append(ident)
        per_core["ones_c_d"].append(ones_c)
        per_core["ones_r_d"].append(ones_r)
        per_core["gain_d"].append(np.ascontiguousarray(gains))
        per_core["invd_d"].append(invd)
        per_core["eps_d"].append(eps_t)
    return {n: np.concatenate(v, axis=0) for n, v in per_core.items()}


def _assemble(res_out, res_v):
    """res_out: (8, T, QW); res_v: (8, 128, NTT, 128) -> (out, v)."""
    out = np.empty((B, T, DIM), np.float32)
    v = np.empty((B, T, HKV, D), np.float32)
    for c in range(N_CORES):
        b, kv = divmod(c, 4)
        out[b, :, kv * QW:(kv + 1) * QW] = res_out[c]
        v[b, :, kv, :] = res_v[c].transpose(1, 0, 2).reshape(T, D)
    return out, v


def _execute(concat_inputs):
    sharded, mesh, in_names, out_names, zero_shapes = _get_runner()
    args = [concat_inputs[n] for n in in_names]
    zeros = [np.zeros((N_CORES * s[0], *s[1:]), dt) for s, dt in zero_shapes]
    outs = sharded(*args, *zeros)
    result = {}
    for i, name in enumerate(out_names):
        arr = np.asarray(outs[i])
        s = zero_shapes[i][0]
        result[name] = arr.reshape(N_CORES, *s)
    return result


def kernel(x, Wq, Wk, Wv, Wproj, q_gain):
    try:
        concat = _prep_inputs(x, Wq, Wk, Wv, Wproj, q_gain)
        result = _execute(concat)
        return _assemble(result["out_sh"], result["v_out"])
    except Exception:
        # fallback: the stock SPMD runner (handles native NRT and axon)
        from concourse.bass_utils import run_bass_kernel_spmd
        maps = _prep_core_maps(x, Wq, Wk, Wv, Wproj, q_gain)
        res = run_bass_kernel_spmd(_get_nc(), maps,
                                   core_ids=list(range(N_CORES)))
        out_sh = np.stack([res.results[c]["out_sh"] for c in range(N_CORES)])
        v_out = np.stack([res.results[c]["v_out"] for c in range(N_CORES)])
        return _assemble(out_sh, v_out)


# ---------------- benchmarking helpers (used by test.py) --------------------

def _build_noop():
    """Tiny NEFF used to estimate per-call dispatch/RPC overhead."""
    import concourse.bacc as bacc
    import concourse.mybir as mybir
    import concourse.tile as tile
    F32 = mybir.dt.float32
    nc = bacc.Bacc("TRN2", target_bir_lowering=False, debug=False,
                   num_devices=N_CORES)
    a = nc.dram_tensor("na", [128, 128], F32, kind="ExternalInput").ap()
    b = nc.dram_tensor("nb", [128, 128], F32, kind="ExternalOutput").ap()
    with tile.TileContext(nc) as tc:
        with tc.tile_pool(name="sb", bufs=1) as sb:
            t = sb.tile([128, 128], F32)
            nc.sync.dma_start(t[:], a[:])
            nc.sync.dma_start(b[:], t[:])
    nc.compile()
    return nc


def _time_runner(runner_tuple, concat_inputs, iters):
    import jax
    from jax.sharding import NamedSharding, PartitionSpec
    import time as _time
    sharded, mesh, in_names, out_names, zero_shapes = runner_tuple
    shard = NamedSharding(mesh, PartitionSpec("core"))
    dev_args = [jax.device_put(concat_inputs[n], shard) for n in in_names]
    zero_sets = [
        [jax.device_put(np.zeros((N_CORES * s[0], *s[1:]), dt), shard)
         for s, dt in zero_shapes]
        for _ in range(iters + 1)
    ]
    outs = sharded(*dev_args, *zero_sets[0])
    jax.block_until_ready(outs)
    times = []
    for i in range(iters):
        t0 = _time.perf_counter()
        outs = sharded(*dev_args, *zero_sets[i + 1])
        jax.block_until_ready(outs)
        times.append(_time.perf_counter() - t0)
    return times


def bench(concat_inputs, iters=8):
    """Returns (best_kernel_s, best_noop_s, all_kernel_times, all_noop_times)."""
    runner = _get_runner()
    kt = _time_runner(runner, concat_inputs, iters)
    noop_runner = _get_runner(nc=_build_noop(), cache_key="noop_runner")
    nt = _time_runner(noop_runner, {"na": np.zeros(
        (N_CORES * 128, 128), np.float32)}, iters)
    return min(kt), min(nt), kt, nt
